# revision 6
# baseline (speedup 1.0000x reference)
"""Trainium2 Bass kernel for nn_ChannelSpatialModulatedConv2d.

Data-parallel over batch across 8 NeuronCores (4 samples each). Per core:
  1. style  = style_chan @ (mod_w*ls).T + mod_b             (PE, fp32)
  2. wsc    = conv_scale * weight * style[b,ci]             (DVE tensor_scalar)
     demod  = rsqrt(sum(wsc^2) over (ci,kk) + eps) per co   (ACT square, DVE
              kk-reduce, PE ones-matmul -> [128co,1] partition-native)
  3. conv2d(x[b], wsc) via 18 accumulating fp32r matmuls per [128co x 512yx]
     PSUM tile (2 ci-tiles x 9 shifts), shifted-window APs over a zero-padded
     66x66 SBUF image.
  4. sp map = style_sp @ (sp_w*ls).T + sp_b, spatially demodulated. The PSUM
     epilogue computes out = (psum * demod[co]) * spmap[yx] in one fused
     scalar_tensor_tensor (demod is NOT folded into the weights; conv is
     linear so this is equivalent).

The baked walrus build only supports ONE sync wait per instruction, so the
Bass subclass rewrites the scheduled BIR JSON, hoisting extra waits onto
single-wait EventSemaphore carriers inserted before the instruction (same
engine => identical blocking semantics).
"""

import json
import sys
from contextlib import ExitStack

for _p in ("/opt/pypackages", "/opt/trn_rl_repo"):
    if _p not in sys.path:
        sys.path.insert(0, _p)

import ml_dtypes
import numpy as np

import concourse.bass as bass
import concourse.mybir as mybir
import concourse.tile as tile
from concourse.tile_rust import add_dep_helper
from concourse.bass_utils import run_bass_kernel_spmd

# Problem constants (hardcoded per harness contract)
B, CIN, COUT, K = 32, 256, 256, 3
STYLE_DIM, SP = 512, 64
EPS = 1e-6
LS = 1.0 / (STYLE_DIM // 2) ** 0.5      # EqualLinear scale = 1/16
CS = 1.0 / (CIN * K * K) ** 0.5         # conv fan-in scale = 1/48
N_CORES = 8
BPC = B // N_CORES                      # samples per core = 4
SPP = SP + 2                            # padded image dim = 66
CKK = COUT * K * K                      # 2304 free columns in weight layout
YX = SP * SP                            # 4096 spatial positions

F32 = mybir.dt.float32
F32R = mybir.dt.float32r
BF16 = mybir.dt.bfloat16
F16 = mybir.dt.float16
AF = mybir.ActivationFunctionType
ALU = mybir.AluOpType


def _split_multi_waits(bir: dict) -> int:
    """Hoist all but one sync wait from every instruction onto single-wait
    EventSemaphore carriers inserted immediately before it (same engine)."""
    ctr = 0
    for fn in bir.get("functions", []):
        for blk in fn.get("blocks", []):
            insts = blk.get("instructions", [])
            if not any(
                len(((i.get("sync_info") or {}).get("on_wait") or [])) > 1
                for i in insts
            ):
                continue
            new_insts = []
            for inst in insts:
                si = inst.get("sync_info")
                ow = (si or {}).get("on_wait") or []
                if len(ow) > 1:
                    for w in ow[:-1]:
                        ctr += 1
                        new_insts.append({
                            "debug": inst.get("debug", 0),
                            "engine": inst["engine"],
                            "ins": [],
                            "outs": [],
                            "name": f"waitsplit-{ctr}",
                            "opcode": "EventSemaphore",
                            "sync_info": {"on_update": [], "on_wait": [w]},
                        })
                    si["on_wait"] = [ow[-1]]
                new_insts.append(inst)
            blk["instructions"] = new_insts
    return ctr


class _WaitSplitBass(bass.Bass):
    def to_json_bytes(self) -> bytes:
        raw = super().to_json_bytes()
        bir = json.loads(raw)
        if _split_multi_waits(bir):
            return json.dumps(bir).encode()
        return raw


def _pbcast(ap, n):
    """Manual 0-step partition broadcast AP (DMA-only; engines reject it)."""
    return bass.AP(tensor=ap.tensor, offset=ap.offset,
                   ap=[[0, n]] + [list(d) for d in ap.ap[1:]])


def _build_program() -> bass.Bass:
    nc = _WaitSplitBass("TRN2", target_bir_lowering=False, debug=False)

    x_d = nc.dram_tensor("x", [BPC, 128, 2, SPP * SPP], BF16, kind="ExternalInput")
    stcT_d = nc.dram_tensor("stcT", [256, BPC], F32, kind="ExternalInput")
    stsT_d = nc.dram_tensor("stsT", [256, BPC], BF16, kind="ExternalInput")
    wT_d = nc.dram_tensor("wT", [CIN, CKK], BF16, kind="ExternalInput")
    mod_wT_d = nc.dram_tensor("mod_wT", [256, CIN], F32, kind="ExternalInput")
    mod_b_d = nc.dram_tensor("mod_b", [CIN, 1], F32, kind="ExternalInput")
    sp_wT_d = nc.dram_tensor("sp_wT", [256, YX], BF16, kind="ExternalInput")
    sp_b_d = nc.dram_tensor("sp_b", [1, YX], F32, kind="ExternalInput")
    out_d = nc.dram_tensor("out", [BPC, COUT, SP, SP], F32, kind="ExternalOutput")
    spm_d = nc.dram_tensor("spm_scratch", [BPC, YX], F16, kind="Internal")
    dspt_d = nc.dram_tensor("dspt_scratch", [BPC, 1], F32, kind="Internal")

    with tile.TileContext(nc) as tc:
        with tc.tile_pool(name="const", bufs=1) as cpool, \
             tc.tile_pool(name="persist", bufs=1) as ppool, \
             tc.tile_pool(name="sps", bufs=1, space="PSUM") as spsum:

            # ---------- constants ----------
            onesF = cpool.tile([128, 2], F32, name="onesF")
            nc.vector.memset(onesF, 1.0)
            ones = cpool.tile([128, 2], F32R, name="ones")
            nc.vector.tensor_copy(ones, onesF)
            eps4 = cpool.tile([BPC, 1], F32, name="eps4")
            nc.vector.memset(eps4, EPS)
            eps128 = cpool.tile([128, 1], F32, name="eps128")
            nc.vector.memset(eps128, EPS)

            # ---------- persistent weights / style ----------
            wt = [ppool.tile([128, CKK], BF16, name=f"wt{k}") for k in range(2)]
            stylec = [ppool.tile([128, BPC], F32, name=f"stylec{k}") for k in range(2)]
            spm = ppool.tile([BPC, YX], F16, name="spm")
            _xp_cm = tc.tile_pool(name="xp", bufs=2)
            xppool = _xp_cm.__enter__()
            _sw_cm = tc.tile_pool(name="swtc", bufs=8)
            swpool = _sw_cm.__enter__()

            def load_xp(xp, b):
                # interleave k0/k1 row-halves so both ci-tiles' early rows
                # land first and conv groups can start while the rest streams
                half = (SPP // 2) * SPP
                for h in range(2):
                    lo = h * half
                    hi = half if h == 0 else SPP * SPP
                    for k in range(2):
                        nc.sync.dma_start(out=xp[k][:, lo:hi],
                                          in_=x_d.ap()[b, :, k, lo:hi])

            # ---------- setup (pool freed afterwards) ----------
            with tc.tile_pool(name="setup", bufs=1) as spool, \
                 tc.tile_pool(name="setup_ps", bufs=1, space="PSUM") as supsum:
                mw = [spool.tile([128, CIN], F32, name=f"mw{k}") for k in range(2)]
                stc = [spool.tile([128, BPC], F32, name=f"stc{k}") for k in range(2)]
                sts = [ppool.tile([128, BPC], BF16, name=f"sts{k}") for k in range(2)]
                mb = [spool.tile([128, 1], F32, name=f"mb{k}") for k in range(2)]
                spb = ppool.tile([BPC, YX], F32, name="spb", tag="spsc")
                scratch = ppool.tile([BPC, YX], F32, name="scratch", tag="spsc")
                # small loads first, on the HWDGE queues which come up
                # before the engine sequencers (style path gates the first
                # conv matmul)
                for k in range(2):
                    nc.sync.dma_start(out=mw[k], in_=mod_wT_d.ap()[k * 128:(k + 1) * 128, :])
                    nc.sync.dma_start(out=stc[k], in_=stcT_d.ap()[k * 128:(k + 1) * 128, :])
                    nc.sync.dma_start(out=sts[k], in_=stsT_d.ap()[k * 128:(k + 1) * 128, :])
                    nc.sync.dma_start(out=mb[k], in_=mod_b_d.ap()[k * 128:(k + 1) * 128, :])
                nc.gpsimd.dma_start(out=spb, in_=_pbcast(sp_b_d.ap(), BPC))
                # then, in critical-path order: conv weights k0, sample-0
                # image first halves, weights k1 (sp_wT and the second image
                # halves follow after the sp-map chunk loads below)
                nc.sync.dma_start(out=wt[0], in_=wT_d.ap()[0:128, :])
                xp0 = [
                    xppool.tile([128, SPP * SPP], BF16, name=f"xp{k}_0", tag=f"xp{k}")
                    for k in range(2)
                ]
                _half = (SPP // 2) * SPP
                for k in range(2):
                    nc.sync.dma_start(out=xp0[k][:, 0:_half],
                                      in_=x_d.ap()[0, :, k, 0:_half])
                nc.sync.dma_start(out=wt[1], in_=wT_d.ap()[128:256, :])

                # channel style: stylec[ci, b] = CS*(mod_w@chan*LS + mod_b)
                for m in range(2):
                    ps_style = supsum.tile([128, BPC], F32, name="ps_style", tag="ps_style")
                    for k in range(2):
                        nc.tensor.matmul(
                            ps_style, mw[k][:, m * 128:(m + 1) * 128], stc[k],
                            start=(k == 0), stop=(k == 1),
                        )
                    mbcs = spool.tile([128, 1], F32, name=f"mbcs{m}")
                    nc.scalar.mul(mbcs, mb[m], CS)
                    nc.scalar.activation(
                        out=stylec[m], in_=ps_style, func=AF.Identity,
                        bias=mbcs, scale=LS * CS,
                    )

                # spatial map: spm[b, yx] = sp_psum*LS + sp_b
                # Per-chunk pipeline: matmul -> (bias+scale, fused square
                # accumulation) -> immediate DRAM staging of the UNSCALED map.
                # The global spatial demod factor is folded into the per-co
                # demod column (dcol) later, so nothing here serializes on the
                # full map.
                sums = ppool.tile([BPC, 8], F32, name="sums")
                sp_mms = []
                for n in range(8):
                    ps_sp = spsum.tile([BPC, 512], F32, name="ps_sp", tag="ps_sp")
                    for k in range(2):
                        swtc = swpool.tile([128, 512], BF16,
                                           name=f"swtc_{n}_{k}", tag="swtc")
                        nc.sync.dma_start(
                            out=swtc,
                            in_=sp_wT_d.ap()[k * 128:(k + 1) * 128,
                                             n * 512:(n + 1) * 512],
                        )
                        sp_mms.append(nc.tensor.matmul(
                            ps_sp, sts[k], swtc,
                            start=(k == 0), stop=(k == 1),
                        ))
                    nc.vector.scalar_tensor_tensor(
                        out=spm[:, n * 512:(n + 1) * 512], in0=ps_sp, scalar=LS,
                        in1=spb[:, n * 512:(n + 1) * 512],
                        op0=ALU.mult, op1=ALU.add,
                    )
                    nc.vector.scalar_tensor_tensor(
                        out=scratch[:, n * 512:(n + 1) * 512],
                        in0=spm[:, n * 512:(n + 1) * 512], scalar=1.0,
                        in1=spm[:, n * 512:(n + 1) * 512],
                        op0=ALU.mult, op1=ALU.mult,
                        accum_out=sums[:, n:n + 1],
                    )
                nc.gpsimd.dma_start(out=spm_d.ap(), in_=spm)

                # global spatial demod scalar: dspt = sqrt(YX/sum + eps)
                ssq = ppool.tile([BPC, 1], F32, name="ssq")
                nc.vector.reduce_sum(out=ssq, in_=sums, axis=mybir.AxisListType.X)
                rsq = ppool.tile([BPC, 1], F32, name="rsq")
                nc.vector.reciprocal(rsq, ssq)
                dspt = ppool.tile([BPC, 1], F32, name="dspt")
                nc.scalar.activation(
                    out=dspt, in_=rsq, func=AF.Sqrt, bias=eps4, scale=float(YX),
                )
                nc.gpsimd.dma_start(out=dspt_d.ap(), in_=dspt)
                for k in range(2):
                    nc.sync.dma_start(out=xp0[k][:, _half:SPP * SPP],
                                      in_=x_d.ap()[0, :, k, _half:SPP * SPP])
                # (second halves land after the sp-map chunks above)

            # ---------- per-sample pipeline ----------
            xtiles = {0: xp0}
            _stack = ExitStack()
            cpsum = _stack.enter_context(tc.tile_pool(name="cps", bufs=6, space="PSUM"))
            wscpool = _stack.enter_context(tc.tile_pool(name="wsc", bufs=2))
            wsqpool = _stack.enter_context(tc.tile_pool(name="wsq", bufs=1))
            dempool = _stack.enter_context(tc.tile_pool(name="dem", bufs=2))
            opool = _stack.enter_context(tc.tile_pool(name="ot", bufs=3))
            smpool = _stack.enter_context(tc.tile_pool(name="smb", bufs=2))

            for b in range(BPC):
                # modulated (pre-demod) weight: wsc = wt * (CS*style[ci,b])
                wsc = [
                    wscpool.tile([128, CKK], BF16, name=f"wsc{k}_{b}", tag=f"wsc{k}")
                    for k in range(2)
                ]
                wsq = [
                    wsqpool.tile([128, CKK], F32, name=f"wsq{k}_{b}", tag="wsq")
                    for k in range(2)
                ]
                wsqk = [
                    dempool.tile([128, COUT], F32R, name=f"wsqk{k}_{b}", tag=f"wsqk{k}")
                    for k in range(2)
                ]
                for k in range(2):
                    nc.vector.tensor_scalar_mul(wsc[k], wt[k], stylec[k][:, b:b + 1])
                    nc.scalar.activation(out=wsq[k], in_=wsc[k],
                                         func=AF.Square)
                    with nc.allow_low_precision(reason="f32r is fp32-width"):
                        nc.vector.reduce_sum(
                            out=wsqk[k],
                            in_=wsq[k].rearrange("p (co kk) -> p co kk", kk=9),
                            axis=mybir.AxisListType.X,
                        )

                # per-sample spatial demod scalar, replicated to 128 partitions
                dsptb = dempool.tile([128, 1], F32, name=f"dsptb_{b}", tag="dsptb")
                nc.gpsimd.dma_start(out=dsptb, in_=_pbcast(dspt_d.ap()[b:b + 1, :], 128))

                # per-co demod, partition-native: ps_d[co,1] = sum_ci wsqk
                dcol = []
                for m in range(2):
                    ps_d = spsum.tile([128, 2], F32, name=f"ps_d_{b}_{m}", tag="ps_d")
                    for k in range(2):
                        nc.tensor.matmul(
                            ps_d, wsqk[k][:, m * 128:(m + 1) * 128], ones,
                            start=(k == 0), stop=(k == 1),
                        )
                    dsq = dempool.tile([128, 1], F32, name=f"dsq_{b}_{m}", tag=f"dsq{m}")
                    nc.scalar.activation(out=dsq, in_=ps_d[:, 0:1], func=AF.Sqrt,
                                         bias=eps128, scale=1.0)
                    dc = dempool.tile([128, 1], F32, name=f"dcol_{b}_{m}", tag=f"dcol{m}")
                    nc.vector.reciprocal(dc, dsq)
                    nc.vector.tensor_mul(dc, dc, dsptb)
                    dcol.append(dc)

                # padded input image [128ci, 66, 66] per ci-tile; prefetch
                # the NEXT sample now so its loads sit ahead of this sample's
                # output stores in the DMA queues (no head-of-line blocking)
                xp = xtiles.pop(b)
                if b + 1 < BPC:
                    nxt = [
                        xppool.tile([128, SPP * SPP], BF16,
                                    name=f"xp{k}_{b + 1}", tag=f"xp{k}")
                        for k in range(2)
                    ]
                    load_xp(nxt, b + 1)
                    xtiles[b + 1] = nxt

                # conv + fused epilogue: out = (psum * demod[co]) * spmap[yx]
                smb = smpool.tile([128, YX], F16, name=f"smb_{b}", tag="smb")
                nc.gpsimd.dma_start(out=smb, in_=_pbcast(spm_d.ap()[b:b + 1, :], 128))
                for n in range(8):
                    for m in range(2):
                        ps = cpsum.tile([128, 512], F32, name=f"ps_{b}_{m}_{n}", tag="ps")
                        i = 0
                        for k in range(2):
                            wv = wsc[k].rearrange("p (co kk) -> p co kk", kk=9)
                            xpv = xp[k].rearrange("p (r c) -> p r c", c=SPP)
                            for s in range(9):
                                dy, dx = s // 3, s % 3
                                mm = nc.tensor.matmul(
                                    ps,
                                    wv[:, m * 128:(m + 1) * 128, s],
                                    xpv[:, n * 8 + dy:n * 8 + dy + 8, dx:dx + SP],
                                    start=(i == 0), stop=(i == 17),
                                )
                                i += 1
                        if b == 0 and n == 0 and m == 0 and sp_mms:
                            # Keep the spatial-map matmuls out of the PE
                            # stream until sample-0 conv has started (their
                            # sp_wT input streams in later; scheduling them
                            # early head-of-line-blocks the PE).
                            for _sp in sp_mms:
                                add_dep_helper(
                                    _sp.ins, mm.ins, sync=False,
                                    reason="sp-map after early sample-0 conv",
                                )
                            sp_mms = []
                        ot = opool.tile([128, 512], F32, name=f"ot_{b}_{m}_{n}", tag="ot")
                        nc.vector.scalar_tensor_tensor(
                            out=ot, in0=ps, scalar=dcol[m][:, 0:1],
                            in1=smb[:, n * 512:(n + 1) * 512],
                            op0=ALU.mult, op1=ALU.mult,
                        )
                        nc.sync.dma_start(
                            out=out_d.ap()[b, m * 128:(m + 1) * 128, n * 8:(n + 1) * 8, :],
                            in_=ot.rearrange("p (r c) -> p r c", c=SP),
                        )
            _stack.close()
            _sw_cm.__exit__(None, None, None)
            _xp_cm.__exit__(None, None, None)
    return nc


_prog_cache = {}


def _get_program() -> bass.Bass:
    if "nc" not in _prog_cache:
        _prog_cache["nc"] = _build_program()
    return _prog_cache["nc"]


def _make_in_maps(inputs):
    x = np.asarray(inputs["x"], dtype=np.float32)
    x = np.pad(x, ((0, 0), (0, 0), (1, 1), (1, 1)))
    style_in = np.asarray(inputs["style_in"], dtype=np.float32)
    weight = np.asarray(inputs["weight"], dtype=np.float32)
    mod_w = np.asarray(inputs["mod_w"], dtype=np.float32)
    mod_b = np.asarray(inputs["mod_b"], dtype=np.float32)
    sp_w = np.asarray(inputs["sp_w"], dtype=np.float32)
    sp_b = np.asarray(inputs["sp_b"], dtype=np.float32)

    # x: [B, 256, SPP, SPP] -> per core [BPC, 128, 2, SPP*SPP] bf16
    x16 = x.astype(ml_dtypes.bfloat16).reshape(B, 2, 128, SPP * SPP) \
        .transpose(0, 2, 1, 3)

    # replicated parameter layouts (transposes/reshapes/dtype casts only)
    wT = np.ascontiguousarray(
        weight[0].transpose(1, 0, 2, 3).reshape(CIN, CKK)).astype(
        ml_dtypes.bfloat16)                                       # [ci, co*9]
    mod_wT = np.ascontiguousarray(mod_w.T)                        # [sd, ci]
    mod_b2 = np.ascontiguousarray(mod_b.reshape(CIN, 1))
    sp_wT = np.ascontiguousarray(sp_w.T).astype(ml_dtypes.bfloat16)
    sp_b2 = np.ascontiguousarray(sp_b.reshape(1, YX))

    in_maps = []
    for c in range(N_CORES):
        sl = slice(c * BPC, (c + 1) * BPC)
        in_maps.append({
            "x": np.ascontiguousarray(x16[sl]),
            "stcT": np.ascontiguousarray(style_in[sl, :256].T),
            "stsT": np.ascontiguousarray(style_in[sl, 256:].T).astype(
                ml_dtypes.bfloat16),
            "wT": wT,
            "mod_wT": mod_wT,
            "mod_b": mod_b2,
            "sp_wT": sp_wT,
            "sp_b": sp_b2,
        })
    return in_maps


def _run(inputs, trace=False):
    nc = _get_program()
    in_maps = _make_in_maps(inputs)
    res = run_bass_kernel_spmd(nc, in_maps, core_ids=list(range(N_CORES)), trace=trace)
    out = np.concatenate([res.results[c]["out"] for c in range(N_CORES)], axis=0)
    return out, res


def kernel(**inputs) -> np.ndarray:
    out, _ = _run(inputs, trace=False)
    return out



# revision 8
# speedup vs baseline: 1.0085x; 1.0085x over previous
"""Trainium2 Bass kernel for nn_ChannelSpatialModulatedConv2d — bf16 conv.

Data-parallel over batch across 8 NeuronCores (4 samples each). Per core:
  1. style  = style_chan @ (mod_w*ls).T + mod_b              (PE, fp32)
  2. wsc    = conv_scale * weight * style[b,ci]  (DVE, bf16 out)
     demod  = rsqrt(sum(wsc^2) + eps) per co    (ACT square, DVE kk-reduce,
              PE ones-matmul -> [128co,1] partition-native)
  3. conv2d(x[b], wsc) via 18 accumulating bf16 matmuls per [128co x 512yx]
     PSUM tile (2 ci-tiles x 9 shifts), shifted-window APs over a zero-padded
     66x66 bf16 SBUF image.
  4. spatial map via bf16 matmuls; map stored fp16, demodulated globally;
     epilogue fuses out = (psum * demod[co]) * spmap[yx] in one DVE
     scalar_tensor_tensor.

DMA economics on this build: every sync dma_start lowers to a serial
DMA_DIRECT2D (~0.6-1.8us sequencer issue each, ~200GB/s aggregate over 16
shared engines), so transfers are aggressively batched: packed style params
(1 DMA), conv weight (1), image halves (1 each), sp_w quarters (4), merged
output pairs, and single-DMA prefetch of later samples via SWDGE.

The sp-map matmuls n0-3 run during the pre-conv PE idle window; n4-7 are
deferred behind the first conv group (their sp_w quarters stream in later)
via add_dep_helper, which keeps them from head-of-line-blocking the PE.

The baked walrus build only supports ONE sync wait per instruction, so the
Bass subclass rewrites the scheduled BIR JSON, hoisting extra waits onto
single-wait EventSemaphore carriers inserted before the instruction.
"""

import json
import sys
from contextlib import ExitStack

for _p in ("/opt/pypackages", "/opt/trn_rl_repo"):
    if _p not in sys.path:
        sys.path.insert(0, _p)

import ml_dtypes
import numpy as np

import concourse.bass as bass
import concourse.mybir as mybir
import concourse.tile as tile
from concourse.tile_rust import add_dep_helper
from concourse.bass_utils import run_bass_kernel_spmd

# Problem constants (hardcoded per harness contract)
B, CIN, COUT, K = 32, 256, 256, 3
STYLE_DIM, SP = 512, 64
EPS = 1e-6
LS = 1.0 / (STYLE_DIM // 2) ** 0.5      # EqualLinear scale = 1/16
CS = 1.0 / (CIN * K * K) ** 0.5         # conv fan-in scale = 1/48
N_CORES = 8
BPC = B // N_CORES                      # samples per core = 4
SPP = SP + 2                            # padded image dim = 66
CKK = COUT * K * K                      # 2304 free columns in weight layout
YX = SP * SP                            # 4096 spatial positions
PIX = SPP * SPP                         # 4356 padded pixels
STY = CIN + BPC + 1                     # packed style-param row: mw|stc|mb

F32 = mybir.dt.float32
F32R = mybir.dt.float32r
BF16 = mybir.dt.bfloat16
F16 = mybir.dt.float16
AF = mybir.ActivationFunctionType
ALU = mybir.AluOpType


def _split_multi_waits(bir: dict) -> int:
    """Hoist all but one sync wait from every instruction onto single-wait
    EventSemaphore carriers inserted immediately before it (same engine)."""
    ctr = 0
    for fn in bir.get("functions", []):
        for blk in fn.get("blocks", []):
            insts = blk.get("instructions", [])
            if not any(
                len(((i.get("sync_info") or {}).get("on_wait") or [])) > 1
                for i in insts
            ):
                continue
            new_insts = []
            for inst in insts:
                si = inst.get("sync_info")
                ow = (si or {}).get("on_wait") or []
                if len(ow) > 1:
                    for w in ow[:-1]:
                        ctr += 1
                        new_insts.append({
                            "debug": inst.get("debug", 0),
                            "engine": inst["engine"],
                            "ins": [],
                            "outs": [],
                            "name": f"waitsplit-{ctr}",
                            "opcode": "EventSemaphore",
                            "sync_info": {"on_update": [], "on_wait": [w]},
                        })
                    si["on_wait"] = [ow[-1]]
                new_insts.append(inst)
            blk["instructions"] = new_insts
    return ctr


class _WaitSplitBass(bass.Bass):
    def to_json_bytes(self) -> bytes:
        raw = super().to_json_bytes()
        bir = json.loads(raw)
        if _split_multi_waits(bir):
            return json.dumps(bir).encode()
        return raw


def _pbcast(ap, n):
    """Manual 0-step partition broadcast AP (DMA-only; engines reject it)."""
    return bass.AP(tensor=ap.tensor, offset=ap.offset,
                   ap=[[0, n]] + [list(d) for d in ap.ap[1:]])


def _build_program() -> bass.Bass:
    nc = _WaitSplitBass("TRN2", target_bir_lowering=False, debug=False)

    x_d = nc.dram_tensor("x", [BPC, 128, 2, PIX], BF16, kind="ExternalInput")
    wT_d = nc.dram_tensor("wT", [128, 2, CKK], BF16, kind="ExternalInput")
    styp_d = nc.dram_tensor("styp", [128, 2, STY], F32, kind="ExternalInput")
    stsT_d = nc.dram_tensor("stsT", [128, 2, BPC], BF16, kind="ExternalInput")
    sp_wT_d = nc.dram_tensor("sp_wT", [128, 2, YX], BF16, kind="ExternalInput")
    sp_b_d = nc.dram_tensor("sp_b", [1, YX], F32, kind="ExternalInput")
    out_d = nc.dram_tensor("out", [BPC, COUT, SP, SP], F32, kind="ExternalOutput")
    spm_d = nc.dram_tensor("spm_scratch", [BPC, YX], F16, kind="Internal")
    dspt_d = nc.dram_tensor("dspt_scratch", [BPC, 1], F32, kind="Internal")

    with tile.TileContext(nc) as tc:
        with tc.tile_pool(name="const", bufs=1) as cpool, \
             tc.tile_pool(name="persist", bufs=1) as ppool, \
             tc.tile_pool(name="sps", bufs=1, space="PSUM") as spsum:

            # ---------- constants ----------
            onesF = cpool.tile([128, 2], F32, name="onesF")
            nc.vector.memset(onesF, 1.0)
            ones = cpool.tile([128, 2], F32R, name="ones")
            nc.vector.tensor_copy(ones, onesF)
            eps4 = cpool.tile([BPC, 1], F32, name="eps4")
            nc.vector.memset(eps4, EPS)
            eps128 = cpool.tile([128, 1], F32, name="eps128")
            nc.vector.memset(eps128, EPS)

            # ---------- persistent tiles ----------
            wt = ppool.tile([128, 2, CKK], BF16, name="wt")
            styp = ppool.tile([128, 2, STY], F32, name="styp")
            sts = ppool.tile([128, 2, BPC], BF16, name="sts")
            swtc = ppool.tile([128, 2, YX], BF16, name="swtc")
            stylec = [ppool.tile([128, BPC], F32, name=f"stylec{k}") for k in range(2)]
            spm = ppool.tile([BPC, YX], F16, name="spm")
            spb = ppool.tile([BPC, YX], F32, name="spb")
            sums = ppool.tile([BPC, 8], F32, name="sums")
            _xp_cm = tc.tile_pool(name="xp", bufs=2)
            xppool = _xp_cm.__enter__()
            _sc_cm = tc.tile_pool(name="scr", bufs=2)
            scpool = _sc_cm.__enter__()

            # ---------- startup loads, critical-path order ----------
            nc.sync.dma_start(out=styp, in_=styp_d.ap())
            nc.sync.dma_start(out=sts, in_=stsT_d.ap())
            nc.sync.dma_start(out=swtc[:, :, 0:2048], in_=sp_wT_d.ap()[:, :, 0:2048])
            nc.sync.dma_start(out=wt, in_=wT_d.ap())
            xp0 = xppool.tile([128, 2, PIX], BF16, name="xp_0", tag="xp")
            _half = (SPP // 2) * SPP
            nc.sync.dma_start(out=xp0[:, :, 0:_half], in_=x_d.ap()[0, :, :, 0:_half])
            nc.gpsimd.dma_start(out=spb, in_=_pbcast(sp_b_d.ap(), BPC))

            with tc.tile_pool(name="setup_ps", bufs=1, space="PSUM") as supsum, \
                 tc.tile_pool(name="setup", bufs=1) as spool:
                # channel style: stylec[ci, b] = CS*(mod_w@chan*LS + mod_b)
                for m in range(2):
                    ps_style = supsum.tile([128, BPC], F32, name="ps_style",
                                           tag="ps_style")
                    for k in range(2):
                        nc.tensor.matmul(
                            ps_style,
                            styp[:, k, m * 128:(m + 1) * 128],
                            styp[:, k, CIN:CIN + BPC],
                            start=(k == 0), stop=(k == 1),
                        )
                    mbcs = spool.tile([128, 1], F32, name=f"mbcs{m}")
                    nc.scalar.mul(mbcs, styp[:, m, CIN + BPC:STY], CS)
                    nc.scalar.activation(
                        out=stylec[m], in_=ps_style, func=AF.Identity,
                        bias=mbcs, scale=LS * CS,
                    )

            def sp_chunk(n):
                """Spatial-map chunk n: 2 matmuls, map stt (DVE), square
                stt (gpsimd, accumulates into sums). Returns the matmuls."""
                mms = []
                ps_sp = spsum.tile([BPC, 512], F32, name=f"ps_sp{n}", tag="ps_sp")
                for k in range(2):
                    mms.append(nc.tensor.matmul(
                        ps_sp, sts[:, k, :], swtc[:, k, n * 512:(n + 1) * 512],
                        start=(k == 0), stop=(k == 1),
                    ))
                nc.vector.scalar_tensor_tensor(
                    out=spm[:, n * 512:(n + 1) * 512], in0=ps_sp, scalar=LS,
                    in1=spb[:, n * 512:(n + 1) * 512],
                    op0=ALU.mult, op1=ALU.add,
                )
                scr = scpool.tile([BPC, 512], F32, name=f"scr{n}", tag="scr")
                nc.vector.scalar_tensor_tensor(
                    out=scr, in0=spm[:, n * 512:(n + 1) * 512], scalar=1.0,
                    in1=spm[:, n * 512:(n + 1) * 512],
                    op0=ALU.mult, op1=ALU.mult,
                    accum_out=sums[:, n:n + 1],
                )
                return mms

            # first half of the spatial map fills the pre-conv PE idle window
            for n in range(4):
                sp_chunk(n)
            nc.gpsimd.dma_start(out=spm_d.ap()[:, 0:2048], in_=spm[:, 0:2048])

            # rest of the big loads
            nc.sync.dma_start(out=xp0[:, :, _half:PIX],
                              in_=x_d.ap()[0, :, :, _half:PIX])
            nc.sync.dma_start(out=swtc[:, :, 2048:4096],
                              in_=sp_wT_d.ap()[:, :, 2048:4096])

            # ---------- per-sample pipeline ----------
            _stack = ExitStack()
            cpsum = _stack.enter_context(tc.tile_pool(name="cps", bufs=6, space="PSUM"))
            wscpool = _stack.enter_context(tc.tile_pool(name="wsc", bufs=2))
            wsqpool = _stack.enter_context(tc.tile_pool(name="wsq", bufs=1))
            dempool = _stack.enter_context(tc.tile_pool(name="dem", bufs=2))
            opool = _stack.enter_context(tc.tile_pool(name="ot", bufs=2))
            smpool = _stack.enter_context(tc.tile_pool(name="smb", bufs=2))

            xtiles = {0: xp0}
            sp_rest = []          # deferred sp-map matmuls (chunks 4..7)
            dspt_done = False

            for b in range(BPC):
                # modulated (pre-demod) weight: wsc = wt * (CS*style[ci,b])
                wsc = wscpool.tile([128, 2, CKK], BF16, name=f"wsc_{b}", tag="wsc")
                for k in range(2):
                    nc.vector.tensor_scalar_mul(
                        wsc[:, k, :], wt[:, k, :], stylec[k][:, b:b + 1])

                if b == 0:
                    # second half of the spatial map + global demod scalar;
                    # emitted after wsc so the DVE reaches the tsm first
                    for n in range(4, 8):
                        sp_rest += sp_chunk(n)
                    nc.gpsimd.dma_start(out=spm_d.ap()[:, 2048:4096],
                                        in_=spm[:, 2048:4096])
                    ssq = ppool.tile([BPC, 1], F32, name="ssq")
                    nc.vector.reduce_sum(out=ssq, in_=sums,
                                         axis=mybir.AxisListType.X)
                    rsq = ppool.tile([BPC, 1], F32, name="rsq")
                    nc.vector.reciprocal(rsq, ssq)
                    dspt = ppool.tile([BPC, 1], F32, name="dspt")
                    nc.scalar.activation(out=dspt, in_=rsq, func=AF.Sqrt,
                                         bias=eps4, scale=float(YX))
                    nc.gpsimd.dma_start(out=dspt_d.ap(), in_=dspt)

                # demod: wsq = wsc^2 (ACT), kk-reduce (DVE), ci ones-matmul
                wsq = wsqpool.tile([128, 2, CKK], F32, name=f"wsq_{b}", tag="wsq")
                nc.scalar.activation(out=wsq, in_=wsc, func=AF.Square)
                wsqk = [
                    dempool.tile([128, COUT], F32R, name=f"wsqk{k}_{b}", tag=f"wsqk{k}")
                    for k in range(2)
                ]
                for k in range(2):
                    with nc.allow_low_precision(reason="f32r is fp32-width"):
                        nc.vector.reduce_sum(
                            out=wsqk[k],
                            in_=wsq[:, k, :].rearrange("p (co kk) -> p co kk", kk=9),
                            axis=mybir.AxisListType.X,
                        )

                # per-sample spatial demod scalar, replicated to 128 partitions
                dsptb = dempool.tile([128, 1], F32, name=f"dsptb_{b}", tag="dsptb")
                nc.gpsimd.dma_start(out=dsptb, in_=_pbcast(dspt_d.ap()[b:b + 1, :], 128))

                # per-co demod column: dcol = dsptb / sqrt(sum_ci wsc^2 + eps)
                dcol = []
                for m in range(2):
                    ps_d = spsum.tile([128, 2], F32, name=f"ps_d_{b}_{m}", tag="ps_d")
                    for k in range(2):
                        nc.tensor.matmul(
                            ps_d, wsqk[k][:, m * 128:(m + 1) * 128], ones,
                            start=(k == 0), stop=(k == 1),
                        )
                    dsq = dempool.tile([128, 1], F32, name=f"dsq_{b}_{m}", tag=f"dsq{m}")
                    nc.scalar.activation(out=dsq, in_=ps_d[:, 0:1], func=AF.Sqrt,
                                         bias=eps128, scale=1.0)
                    dc = dempool.tile([128, 1], F32, name=f"dcol_{b}_{m}", tag=f"dcol{m}")
                    nc.vector.reciprocal(dc, dsq)
                    nc.vector.tensor_mul(dc, dc, dsptb)
                    dcol.append(dc)

                # image for this sample; prefetch the next one whole (SWDGE,
                # off the sync sequencer's serial DIRECT2D path)
                xp = xtiles.pop(b)
                if b + 1 < BPC:
                    nxt = xppool.tile([128, 2, PIX], BF16, name=f"xp_{b+1}", tag="xp")
                    nc.gpsimd.dma_start(out=nxt, in_=x_d.ap()[b + 1])
                    xtiles[b + 1] = nxt

                # spatial-map broadcast for this sample, in halves (fp16)
                smb = [smpool.tile([128, 2048], F16, name=f"smb_{b}_{h}", tag=f"smb{h}")
                       for h in range(2)]
                for h in range(2):
                    nc.gpsimd.dma_start(
                        out=smb[h],
                        in_=_pbcast(spm_d.ap()[b:b + 1, h * 2048:(h + 1) * 2048], 128))

                # conv + fused epilogue: out = (psum * demod[co]) * spmap[yx]
                xpv = xp.rearrange("p h (r c) -> p h r c", c=SPP)
                wv = wsc.rearrange("p h (co kk) -> p h co kk", kk=9)
                ot_t = {}
                for n in range(8):
                    for m in range(2):
                        ps = cpsum.tile([128, 512], F32, name=f"ps_{b}_{m}_{n}", tag="ps")
                        i = 0
                        for k in range(2):
                            for s in range(9):
                                dy, dx = s // 3, s % 3
                                mm = nc.tensor.matmul(
                                    ps,
                                    wv[:, k, m * 128:(m + 1) * 128, s],
                                    xpv[:, k, n * 8 + dy:n * 8 + dy + 8, dx:dx + SP],
                                    start=(i == 0), stop=(i == 17),
                                )
                                i += 1
                        if b == 0 and n == 0 and m == 0 and sp_rest:
                            # keep deferred sp-map matmuls from head-of-line
                            # blocking the first conv group
                            for _sp in sp_rest:
                                add_dep_helper(
                                    _sp.ins, mm.ins, sync=False,
                                    reason="sp-map after early sample-0 conv",
                                )
                            sp_rest = []
                        if n % 2 == 0:
                            ot_t[m] = opool.tile([128, 1024], F32,
                                                 name=f"ot_{b}_{m}_{n}", tag=f"ot{m}")
                        ot = ot_t[m]
                        half = (n % 2) * 512
                        nc.vector.scalar_tensor_tensor(
                            out=ot[:, half:half + 512], in0=ps,
                            scalar=dcol[m][:, 0:1],
                            in1=smb[n // 4][:, (n % 4) * 512:(n % 4) * 512 + 512],
                            op0=ALU.mult, op1=ALU.mult,
                        )
                        if n % 2 == 1:
                            nc.sync.dma_start(
                                out=out_d.ap()[b, m * 128:(m + 1) * 128,
                                               (n - 1) * 8:(n + 1) * 8, :],
                                in_=ot.rearrange("p (r c) -> p r c", c=SP),
                            )
            _stack.close()
            _sc_cm.__exit__(None, None, None)
            _xp_cm.__exit__(None, None, None)
    return nc


_prog_cache = {}


def _get_program() -> bass.Bass:
    if "nc" not in _prog_cache:
        _prog_cache["nc"] = _build_program()
    return _prog_cache["nc"]


def _make_in_maps(inputs):
    x = np.asarray(inputs["x"], dtype=np.float32)
    x = np.pad(x, ((0, 0), (0, 0), (1, 1), (1, 1)))
    style_in = np.asarray(inputs["style_in"], dtype=np.float32)
    weight = np.asarray(inputs["weight"], dtype=np.float32)
    mod_w = np.asarray(inputs["mod_w"], dtype=np.float32)
    mod_b = np.asarray(inputs["mod_b"], dtype=np.float32)
    sp_w = np.asarray(inputs["sp_w"], dtype=np.float32)
    sp_b = np.asarray(inputs["sp_b"], dtype=np.float32)

    # x: [B, 256, SPP, SPP] -> per core [BPC, 128, 2, PIX] bf16
    x16 = x.astype(ml_dtypes.bfloat16).reshape(B, 2, 128, PIX).transpose(0, 2, 1, 3)

    # replicated parameters (transposes/reshapes/dtype casts only)
    wT = np.ascontiguousarray(
        weight[0].transpose(1, 0, 2, 3).reshape(2, 128, CKK).transpose(1, 0, 2)
    ).astype(ml_dtypes.bfloat16)                                  # [p, k, co*9]
    sp_wT = np.ascontiguousarray(
        sp_w.T.reshape(2, 128, YX).transpose(1, 0, 2)).astype(ml_dtypes.bfloat16)
    sp_b2 = np.ascontiguousarray(sp_b.reshape(1, YX))
    mod_wT = mod_w.T.reshape(2, 128, CIN)                          # [k, p, ci]

    in_maps = []
    for c in range(N_CORES):
        sl = slice(c * BPC, (c + 1) * BPC)
        # packed style params: [128, 2, mw(256) | stc(4) | mb(1)] fp32
        styp = np.empty((128, 2, STY), dtype=np.float32)
        styp[:, :, :CIN] = mod_wT.transpose(1, 0, 2)
        styp[:, :, CIN:CIN + BPC] = style_in[sl, :256].T.reshape(2, 128, BPC) \
            .transpose(1, 0, 2)
        styp[:, :, CIN + BPC] = mod_b.reshape(2, 128).T
        stsT = style_in[sl, 256:].T.reshape(2, 128, BPC).transpose(1, 0, 2) \
            .astype(ml_dtypes.bfloat16)
        in_maps.append({
            "x": np.ascontiguousarray(x16[sl]),
            "wT": wT,
            "styp": styp,
            "stsT": np.ascontiguousarray(stsT),
            "sp_wT": sp_wT,
            "sp_b": sp_b2,
        })
    return in_maps


def _run(inputs, trace=False):
    nc = _get_program()
    in_maps = _make_in_maps(inputs)
    res = run_bass_kernel_spmd(nc, in_maps, core_ids=list(range(N_CORES)), trace=trace)
    out = np.concatenate([res.results[c]["out"] for c in range(N_CORES)], axis=0)
    return out, res


def kernel(**inputs) -> np.ndarray:
    out, _ = _run(inputs, trace=False)
    return out
